# revision 26
# baseline (speedup 1.0000x reference)
"""Trainium2 Bass kernel for the Dirichlet-KDE ECE loss (nn_KDEECE).

reference math (N=8192, C=10, h=0.1):
  f        = softmax(logits)                      [N, C]
  alphas   = f/h + 1
  log_beta = sum_c lgamma(alphas) - lgamma(20)    [N]
  log_kern = log(f+eps) @ (10 f).T - log_beta[j], diag = -inf
  kern     = exp(log_kern)
  out      = mean_i sum_c | (kern @ onehot)/rowsum(kern) - f |

Device strategy (8 cores, i rows sharded 1024/core, all of j on each core):
  Columns j are sorted by label so each class is a contiguous segment.
  A single PE pass computes, in psum,  y = A_S * log_kern + B_S  with the
  affine folded into the matmul weights (L' = [A_S*log f | 1],
  A' = [10 f | -A_S*log_beta + B_S]).  The elementwise exp and the
  per-class row sums are then distributed over ACT / DVE / POOL:

  * converter ACT: activation(Exp, scale=1/A_S, bias=-B_S/A_S) - exact
    exp from psum; with accum_out it also yields the class sum for free.
  * converter DVE: one tensor_scalar  max(y, -32768) -> int16 whose bit
    pattern IS bf16(exp(log_kern)) (Schraudolph fast exp; B_S carries a
    mean-zero correction; class sums land ~1e-3 accurate).
  * reducer DVE: grouped tensor_reduce over uniform-width class tiles
    (bank partials summed on host).
  * reducer POOL: gpsimd tensor_scalar(+0, accum_out) over the SBUF bf16
    class tile (GPSIMD cannot read PSUM, so it only ever reduces).
  * baseline-layout tiles ([128 j x 1024 i], sorted prefix columns):
    exp by ACT/DVE from psum, class sums by PE onehot matmuls into a
    psum accumulator - this uses PE's spare cycles.

  The leave-one-out diagonal is subtracted on host via the analytic
  self-term kern_ii.  Host work is O(N*C).
"""

import numpy as np

N, C = 8192, 10
H_INV = 10.0
EPS_LOG = 1e-45
EPS_DEN = 1e-10
N_CORES = 8
LOC = N // N_CORES  # 1024 i rows per core
IT = LOC // 128  # 8 i-tiles per core
CK = C + 1  # contraction rows: 10 A-rows + bias row

A_S = 128.0 / np.log(2.0)  # 184.6617...
# 16256 maps exp(0) -> bf16 1.0; the -6.71 makes the Schraudolph per-element
# error mean-zero (-0.0563 linear-mantissa bias + 0.0039 truncation, in
# 1/128-of-log2 units), so per-class sums of ~800 terms are ~1e-3 accurate.
B_S = 16256.0 - 6.71
DEAD = -1.0e9  # bias-row value for pad columns -> exp == 0

# --- tunables -------------------------------------------------------------
J_BASE_TILES = 8  # baseline-layout j-tiles (128 cols each) reduced on PE
WARMUP = 0  # PE p-state warmup matmuls (ramp costs less than the serial warmup chain)
FPA_BUFS = 2  # ACT psum ring (2 banks each)
FPD_BUFS = 2  # DVE psum ring (1 bank each)
BP_BUFS = 1  # baseline psum ring (2 banks each)
SCR_BUFS = 3  # ACT scratch ring
KT_BUFS = 3  # kern tile ring (per reducer group)
KB_BUFS = 4  # baseline kern tile ring

_cache = {}

# combos: (converter, reducer). FOLD = gpsimd tensor_tensor adds the two
# psum-bank halves of a class (halving DVE's reduce volume), DVE reduces.
COMBOS = [
    ("ACT", "FUSED"),
    ("ACT", "DVE"),
    ("ACT", "FOLD"),
    ("DVE", "DVE"),
    ("DVE", "FOLD"),
]


def _lgamma(x):
    try:
        from scipy.special import gammaln

        return gammaln(x)
    except Exception:
        import math

        return np.vectorize(math.lgamma)(x.astype(np.float64))


def _host_prep(logits, labels):
    logits = np.asarray(logits, np.float32)
    labels = np.asarray(labels).astype(np.int64)
    x = logits - logits.max(axis=1, keepdims=True)
    e = np.exp(x)
    f = (e / e.sum(axis=1, keepdims=True)).astype(np.float32)

    alphas = (f.astype(np.float64) * H_INV) + 1.0
    log_beta = _lgamma(alphas).sum(axis=1) - _lgamma(np.full(N, C + H_INV))
    L = np.log(f.astype(np.float64) + EPS_LOG)
    A = (H_INV * f.astype(np.float64)).astype(np.float32)

    kii = np.exp((L * A.astype(np.float64)).sum(axis=1) - log_beta)
    return f, labels, L, A, log_beta, kii


# calibrated per-instruction engine-busy costs (TimelineSim probes), ns
def _combo_cost(u, conv, red):
    """Engine -> ns for one flipped class instance (u = 2w units)."""
    d = {}
    if conv == "ACT":
        d["ACT"] = 0.8333 * u + 185.0 + (187.0 if red == "FUSED" else 0.0)
    else:
        d["DVE"] = 1.0417 * u + 320.0  # two per-bank instructions
    if red == "DVE":
        d["DVE"] = d.get("DVE", 0.0) + 1.0417 * u + 60.0  # grouped, amortized
    elif red == "FOLD":
        d["POOL"] = 0.9921 * u + 60.0  # one grouped add, eff 0.42, amortized
        d["DVE"] = d.get("DVE", 0.0) + 0.5208 * u + 40.0
    return d


def _wseg(cnt):
    # psum-bank half-width: ceil(cnt/2), rounded up to a multiple of 4
    # (fp32r matmul ISA restriction on the moving free size)
    return (((cnt + 1) // 2) + 3) & ~3


def _plan(labels):
    """Sorted-class layout + engine routing."""
    perm = np.argsort(labels, kind="stable")
    sl = labels[perm]
    counts = np.bincount(sl, minlength=C)
    starts = np.concatenate([[0], np.cumsum(counts)])[:C].astype(int)

    b_cols = 128 * J_BASE_TILES  # baseline-layout raw columns (sorted prefix)

    segs = []  # (class, src_off, cnt): flipped part of each class
    for c in range(C):
        s, e = starts[c], starts[c] + int(counts[c])
        s2 = max(s, b_cols)
        if e > s2:
            segs.append((c, s2, e - s2))

    # global routing: local search over combo assignments, balancing
    # ACT/DVE/POOL (PE's ln+ky load is fixed by J_BASE_TILES, not routed)
    nseg = len(segs)
    us = [2.0 * _wseg(segs[i][2]) for i in range(nseg)]
    nbank = 2 * J_BASE_TILES

    OFFS = {"ACT": 0.0, "DVE": 2500.0, "POOL": 6000.0}

    def loads_of(route, base_route):
        load = {"ACT": 0.0, "DVE": 0.0, "POOL": 0.0}
        for i in range(nseg):
            for e, v in _combo_cost(us[i], *route[i]).items():
                load[e] += v * IT
        for e in base_route:
            if e == "ACT":
                load["ACT"] += 0.8333 * 512 + 185.0
            else:
                load["DVE"] += 1.0417 * 512 + 160.0
        return load

    import itertools
    best_route, best_base, best_m = None, None, None
    rng = np.random.RandomState(0)
    for trial in range(6):
        if trial == 0:
            route = [("ACT", "FUSED")] * nseg
        else:
            route = [COMBOS[rng.randint(len(COMBOS))] for _ in range(nseg)]
        base_route = ["ACT" if k % 2 else "DVE" for k in range(2 * J_BASE_TILES)]
        for _ in range(400):
            load = loads_of(route, base_route)
            m = max(load[e] + OFFS[e] for e in load)
            improved = False
            # try flipping one class combo
            for i in range(nseg):
                cur = route[i]
                for cand in COMBOS:
                    if cand == cur:
                        continue
                    route[i] = cand
                    l2 = loads_of(route, base_route)
                    m2 = max(l2[e] + OFFS[e] for e in l2)
                    if m2 < m - 1.0:
                        m = m2
                        improved = True
                        cur = cand
                    else:
                        route[i] = cur
            # try flipping one baseline bank conv
            for k in range(2 * J_BASE_TILES):
                cur = base_route[k]
                base_route[k] = "DVE" if cur == "ACT" else "ACT"
                l2 = loads_of(route, base_route)
                m2 = max(l2[e] + OFFS[e] for e in l2)
                if m2 < m - 1.0:
                    m = m2
                    improved = True
                else:
                    base_route[k] = cur
            if not improved:
                break
        if best_m is None or m < best_m:
            best_route, best_base, best_m = list(route), list(base_route), m
    route = best_route
    base_route = best_base
    load = loads_of(route, base_route)

    # reducer groups
    g_dve = [i for i in range(len(segs)) if route[i][1] == "DVE"]
    g_fold = [i for i in range(len(segs)) if route[i][1] == "FOLD"]
    g_fused = [i for i in range(len(segs)) if route[i][1] == "FUSED"]
    w_dve = max((_wseg(segs[i][2]) for i in g_dve), default=0)
    w_fold = max((_wseg(segs[i][2]) for i in g_fold), default=0)

    # device column layout: [baseline raw | FUSED | FOLD grp | DVE-red grp]
    # (FUSED = ACT's classes go in the first DMA chunk so ACT starts early)
    seg_dev = {}
    pos = b_cols
    for i in g_fused:
        w = _wseg(segs[i][2])
        seg_dev[i] = (pos, int(w))
        pos += 2 * int(w)
    half_hint = pos
    for i in g_fold:
        seg_dev[i] = (pos, int(w_fold))
        pos += 2 * int(w_fold)
        if i == (g_fold[0] if g_fold else None):
            half_hint = pos
    for i in g_dve:
        seg_dev[i] = (pos, int(w_dve))
        pos += 2 * int(w_dve)

    return dict(
        perm=perm, segs=segs, route=route, seg_dev=seg_dev, dev_w=pos,
        b_cols=b_cols, base_route=base_route, load=load,
        g_dve=g_dve, g_fold=g_fold, g_fused=g_fused, w_dve=int(w_dve),
        w_fold=int(w_fold), half_hint=half_hint,
    )


def _build(plan):
    import concourse.bacc as bacc
    import concourse.mybir as mybir
    import concourse.tile as tile

    f32 = mybir.dt.float32
    f32r = mybir.dt.float32r
    bf16 = mybir.dt.bfloat16
    i16 = mybir.dt.int16
    AF = mybir.ActivationFunctionType
    ALU = mybir.AluOpType
    AX = mybir.AxisListType

    segs = plan["segs"]
    seg_dev, dev_w = plan["seg_dev"], plan["dev_w"]
    route, base_route = plan["route"], plan["base_route"]
    g_dve, g_fold, g_fused = plan["g_dve"], plan["g_fold"], plan["g_fused"]
    w_dve, w_fold = plan["w_dve"], plan["w_fold"]
    n_dve, n_fold = len(g_dve), len(g_fold)
    JB = J_BASE_TILES
    nred = max(1, 2 * n_dve + n_fold)
    inv_s = float(1.0 / A_S)
    bias_s = float(-B_S / A_S)
    order = g_dve + g_fold + g_fused

    nc = bacc.Bacc(
        "TRN2",
        target_bir_lowering=False,
        debug=False,
        enable_asserts=False,
        num_devices=N_CORES,
    )
    ap_d = nc.dram_tensor("apT", [CK, dev_w], f32r, kind="ExternalInput")
    lp_d = nc.dram_tensor("lpT", [CK, LOC], f32r, kind="ExternalInput")
    acc_d = nc.dram_tensor("acc", [128, IT * C], f32, kind="ExternalOutput")
    red_d = nc.dram_tensor("red", [128, IT * nred], f32, kind="ExternalOutput")
    names = dict(apT=ap_d.name, lpT=lp_d.name, acc=acc_d.name, red=red_d.name)
    if JB:
        yo_d = nc.dram_tensor("yone", [128, JB * C], bf16, kind="ExternalInput")
        ky_d = nc.dram_tensor("ky", [C, LOC], f32, kind="ExternalOutput")
        names["yone"] = yo_d.name
        names["ky"] = ky_d.name

    # (bt, half) pairs: halves split into program phases so the 1-bank ky
    # accumulator is reused (h=0 finishes, copies out, h=1 restarts); both
    # phases end early so the tail step is pure flipped work
    base_sched = [[] for _ in range(IT)]
    # consecutive bt ranges per step: psum accumulation (start at bt 0,
    # stop at bt JB-1) must execute in emission order
    for bt in range(JB):
        base_sched[1 + (bt * 2) // max(1, JB)].append((bt, 0))
    for bt in range(JB):
        base_sched[3 + (bt * 3) // max(1, JB)].append((bt, 1))

    with tile.TileContext(nc) as tc:
        with (
            tc.tile_pool(name="const", bufs=1) as cp,
            tc.tile_pool(name="scr", bufs=SCR_BUFS) as scp,
            tc.tile_pool(name="ktd", bufs=KT_BUFS) as ktdp,
            tc.tile_pool(name="ktp", bufs=KT_BUFS) as ktpp,
            tc.tile_pool(name="kb", bufs=KB_BUFS) as kbp,
            tc.tile_pool(name="fpa", bufs=FPA_BUFS, space="PSUM") as fpa,
            tc.tile_pool(name="fpd", bufs=FPD_BUFS, space="PSUM") as fpd,
        ):
            ap_sb = cp.tile([CK, dev_w], f32r)
            lp_sb = cp.tile([CK, LOC], f32r)
            garb = cp.tile([CK, 512], f32r)
            acc_sb = cp.tile([128, IT * C], f32)
            red_sb = cp.tile([128, IT * nred], f32)
            bias_sb = cp.tile([128, 1], f32)
            nc.vector.memset(bias_sb[:], bias_s)
            nc.vector.memset(garb[:].bitcast(f32), 0)
            nc.vector.memset(acc_sb[:], 0)
            nc.vector.memset(red_sb[:], 0)
            half = min(dev_w - 128, plan["half_hint"] + 127 & ~127)
            b_cols = plan["b_cols"]
            # parallel queues (SP / ACT / gpsimd SWDGE); the FUSED region
            # (ACT's classes, laid out right after the baseline prefix) goes
            # first so ACT starts converting as early as possible
            nc.scalar.dma_start(lp_sb[:], lp_d.ap())
            nc.sync.dma_start(ap_sb[:, b_cols:half], ap_d.ap()[:, b_cols:half])
            if b_cols:
                nc.sync.dma_start(ap_sb[:, 0:b_cols], ap_d.ap()[:, 0:b_cols])
            nc.gpsimd.dma_start(ap_sb[:, half:dev_w], ap_d.ap()[:, half:dev_w])
            if JB:
                yo_sb = cp.tile([128, JB * C], bf16)
                nc.sync.dma_start(yo_sb[:], yo_d.ap())

            def emit(ky_ps, bpool, ky_sb):
                # merged per-step work list, round-robin across conv engines
                def step_items(t):
                    by_eng = {"ACT": [], "DVE": []}
                    for i in order:
                        by_eng[route[i][0]].append(("seg", i))
                    for bt, h in base_sched[t]:
                        by_eng[base_route[2 * bt + h]].append(("base", (bt, h)))
                    na, nd = len(by_eng["ACT"]), len(by_eng["DVE"])
                    merged, ia, idv = [], 0, 0
                    for _ in range(na + nd):
                        if ia >= na or (idv < nd and idv * na <= ia * nd):
                            merged.append(by_eng["DVE"][idv])
                            idv += 1
                        else:
                            merged.append(by_eng["ACT"][ia])
                            ia += 1
                    return merged

                for t in range(IT):
                    lw = lp_sb[:, t * 128 : (t + 1) * 128]
                    kt_d = (
                        ktdp.tile([128, 2 * n_dve, w_dve], bf16,
                                  tag="ktd", name=f"ktd{t}")
                        if n_dve else None
                    )
                    kt_f = (
                        ktpp.tile([128, 2, n_fold, w_fold], bf16,
                                  tag="ktf", name=f"ktf{t}")
                        if n_fold else None
                    )
                    kf = (
                        ktpp.tile([128, n_fold, w_fold], bf16,
                                  tag="kf", name=f"kf{t}")
                        if n_fold else None
                    )
                    for kind, item in step_items(t):
                        if kind == "base":
                            bt, h = item
                            aw = ap_sb[:, bt * 128 : (bt + 1) * 128]
                            yw = yo_sb[:, bt * C : (bt + 1) * C]
                            bs = bpool.tile([128, 512], f32, tag="bp",
                                            name=f"bp{bt}_{h}")
                            nc.tensor.matmul(
                                bs[:], aw, lp_sb[:, h * 512 : (h + 1) * 512],
                                start=True, stop=True,
                            )
                            kb = kbp.tile([128, 512], bf16, tag="kb",
                                          name=f"kb{bt}_{h}")
                            if base_route[2 * bt + h] == "ACT":
                                nc.scalar.activation(
                                    kb[:], bs[:], AF.Exp,
                                    bias=bias_sb[:], scale=inv_s,
                                )
                            else:
                                nc.vector.tensor_scalar(
                                    kb[:].bitcast(i16), bs[:],
                                    -32768.0, None, ALU.max,
                                )
                            nc.tensor.matmul(
                                ky_ps[:], yw, kb[:],
                                start=(bt == 0), stop=(bt == JB - 1),
                            )
                            if bt == JB - 1:
                                nc.vector.tensor_copy(
                                    ky_sb[:, h * 512 : (h + 1) * 512], ky_ps[:]
                                )
                                nc.sync.dma_start(
                                    ky_d.ap()[:, h * 512 : (h + 1) * 512],
                                    ky_sb[:, h * 512 : (h + 1) * 512],
                                )
                            continue
                        i = item
                        c, off, cnt = segs[i]
                        dev_off, w = seg_dev[i]
                        conv, red = route[i]
                        if conv == "ACT":
                            ps = fpa.tile([128, 2, 512], f32, tag="fpa",
                                          name=f"fpa{t}_{i}")
                            for h in range(2):
                                nc.tensor.matmul(
                                    ps[:, h, 0:w], lw,
                                    ap_sb[:, dev_off + h * w :
                                           dev_off + (h + 1) * w],
                                    start=True, stop=True,
                                )
                            if red == "DVE":
                                s = g_dve.index(i)
                                dst = kt_d[:, 2 * s : 2 * s + 2, 0:w]
                            elif red == "FOLD":
                                s = g_fold.index(i)
                                dst = kt_f[:, :, s, 0:w]
                            else:
                                sc = scp.tile([128, 2, 512], bf16, tag="scr",
                                              name=f"sc{t}_{i}")
                                dst = sc[:, :, 0:w]
                            nc.scalar.activation(
                                dst, ps[:, :, 0:w], AF.Exp,
                                bias=bias_sb[:], scale=inv_s,
                                accum_out=(
                                    acc_sb[:, t * C + c : t * C + c + 1]
                                    if red == "FUSED" else None
                                ),
                            )
                            if red == "FOLD":
                                s = g_fold.index(i)
                                nc.gpsimd.tensor_tensor(
                                    kf[:, s, 0:w], kt_f[:, 0, s, 0:w],
                                    kt_f[:, 1, s, 0:w], ALU.add,
                                )
                        else:
                            # DVE converter: per-bank psum + per-bank convs so
                            # the DVE ring never couples to ACT's pacing
                            for h in range(2):
                                psb = fpd.tile([128, 512], f32, tag="fpd",
                                               name=f"fpd{t}_{i}_{h}")
                                nc.tensor.matmul(
                                    psb[:, 0:w], lw,
                                    ap_sb[:, dev_off + h * w :
                                           dev_off + (h + 1) * w],
                                    start=True, stop=True,
                                )
                                if red == "DVE":
                                    s = g_dve.index(i)
                                    dst = kt_d[:, 2 * s + h, 0:w]
                                else:
                                    s = g_fold.index(i)
                                    dst = kt_f[:, h, s, 0:w]
                                nc.vector.tensor_scalar(
                                    dst.bitcast(i16), psb[:, 0:w],
                                    -32768.0, None, ALU.max,
                                )
                            if red == "FOLD":
                                s = g_fold.index(i)
                                nc.gpsimd.tensor_tensor(
                                    kf[:, s, 0:w], kt_f[:, 0, s, 0:w],
                                    kt_f[:, 1, s, 0:w], ALU.add,
                                )

                    if n_fold:
                        nc.vector.tensor_reduce(
                            red_sb[:, t * nred + 2 * n_dve :
                                   t * nred + 2 * n_dve + n_fold],
                            kf[:], AX.X, ALU.add,
                        )
                    if n_dve:
                        nc.vector.tensor_reduce(
                            red_sb[:, t * nred : t * nred + 2 * n_dve],
                            kt_d[:], AX.X, ALU.add,
                        )
                    if t == IT // 2 - 1:
                        # first-half outputs leave early, shrinking the tail
                        nc.sync.dma_start(
                            acc_d.ap()[:, 0 : (IT // 2) * C],
                            acc_sb[:, 0 : (IT // 2) * C],
                        )
                        nc.sync.dma_start(
                            red_d.ap()[:, 0 : (IT // 2) * nred],
                            red_sb[:, 0 : (IT // 2) * nred],
                        )

            if JB:
                with (
                    tc.tile_pool(name="ky", bufs=1, space="PSUM") as kyp,
                    tc.tile_pool(name="bp", bufs=BP_BUFS, space="PSUM") as bpool,
                ):
                    ky_ps = kyp.tile([C, 512], f32)
                    ky_sb = cp.tile([C, LOC], f32)
                    # PE p-state warmup on garbage weights: overlaps the input
                    # DMAs, WAW-chained on the (later-reset) ky bank so the
                    # flipped psum ring stays free for real work.
                    for wi in range(WARMUP):
                        nc.tensor.matmul(
                            ky_ps[:, 0:256], garb[:, 0:C], garb[:, 0:256],
                            start=True, stop=True,
                        )
                    emit(ky_ps, bpool, ky_sb)
            else:
                emit(None, None, None)

            hc = (IT // 2) * C
            hr = (IT // 2) * nred
            nc.sync.dma_start(acc_d.ap()[:, hc:], acc_sb[:, hc:])
            nc.sync.dma_start(red_d.ap()[:, hr:], red_sb[:, hr:])

    nc.compile()
    return nc, names


def _prep_device_inputs(plan, L, A, log_beta, labels):
    import ml_dtypes

    perm = plan["perm"]
    dev_w, b_cols = plan["dev_w"], plan["b_cols"]
    segs, seg_dev = plan["segs"], plan["seg_dev"]

    Ls = (A_S * L).astype(np.float32)
    bias_row = (-A_S * log_beta + B_S).astype(np.float32)

    As = A[perm]
    brs = bias_row[perm]

    apT = np.zeros((CK, dev_w), np.float32)
    apT[CK - 1, :] = DEAD
    if b_cols:
        apT[:C, 0:b_cols] = As[0:b_cols].T
        apT[CK - 1, 0:b_cols] = brs[0:b_cols]
    for i, (c, off, cnt) in enumerate(segs):
        dev_off, w = seg_dev[i]
        apT[:C, dev_off : dev_off + cnt] = As[off : off + cnt].T
        apT[CK - 1, dev_off : dev_off + cnt] = brs[off : off + cnt]

    lpT = np.concatenate([Ls, np.ones((N, 1), np.float32)], axis=1).T.copy()

    yone = None
    if b_cols:
        sl = labels[perm][:b_cols]
        JB = J_BASE_TILES
        y = np.zeros((b_cols, C), np.float32)
        y[np.arange(b_cols), sl] = 1.0
        yone = (
            y.reshape(JB, 128, C).transpose(1, 0, 2).reshape(128, JB * C)
        ).astype(ml_dtypes.bfloat16)
    return apT, lpT, yone


def kernel(logits, labels):
    from concourse import bass_utils

    f, labels_i, L, A, log_beta, kii = _host_prep(logits, labels)

    key = labels_i.tobytes()
    if key not in _cache:
        plan = _plan(labels_i)
        nc, names = _build(plan)
        _cache.clear()
        _cache[key] = (plan, nc, names)
    plan, nc, names = _cache[key]

    apT, lpT, yone = _prep_device_inputs(plan, L, A, log_beta, labels_i)

    in_maps = []
    for d in range(N_CORES):
        m = {
            names["apT"]: apT,
            names["lpT"]: lpT[:, d * LOC : (d + 1) * LOC].copy(),
        }
        if yone is not None:
            m[names["yone"]] = yone
        in_maps.append(m)
    res = bass_utils.run_bass_kernel_spmd(nc, in_maps, core_ids=list(range(N_CORES)))

    segs = plan["segs"]
    g_dve, g_fold, g_fused = plan["g_dve"], plan["g_fold"], plan["g_fused"]
    n_dve, n_fold = len(g_dve), len(g_fold)
    nred = max(1, 2 * n_dve + n_fold)

    sums = np.zeros((N, C), np.float64)
    for d in range(N_CORES):
        r = res.results[d]
        acc = np.asarray(r[names["acc"]], np.float64).reshape(128, IT, C)
        red = np.asarray(r[names["red"]], np.float64).reshape(128, IT, nred)
        rows = d * LOC + np.arange(LOC).reshape(IT, 128)
        for t in range(IT):
            idx = rows[t]
            for s, i in enumerate(g_dve):
                c = segs[i][0]
                sums[idx, c] += red[:, t, 2 * s] + red[:, t, 2 * s + 1]
            for s, i in enumerate(g_fold):
                c = segs[i][0]
                sums[idx, c] += red[:, t, 2 * n_dve + s]
            for i in g_fused:
                c = segs[i][0]
                sums[idx, c] += acc[:, t, c]
        if J_BASE_TILES:
            ky = np.asarray(r[names["ky"]], np.float64)  # [C, LOC]
            sums[d * LOC : (d + 1) * LOC] += ky.T

    sums[np.arange(N), labels_i] -= kii
    den = np.maximum(sums.sum(axis=1), EPS_DEN)
    ratio = sums / den[:, None]
    per_sample = np.abs(ratio - f.astype(np.float64)).sum(axis=1)
    return np.asarray(per_sample.mean(), dtype=np.float32)


def compiled_nc():
    """The compiled Bacc module of the last kernel() call (for timing)."""
    ((plan, nc, names),) = _cache.values()
    return nc
